# revision 3
# baseline (speedup 1.0000x reference)
"""Strided (residue-group) attention for Trainium2, SPMD across 8 NeuronCores.

Problem: x[B=2,S=4096,E=1024] -> qkv proj -> per-(batch,head,residue-group)
attention (stride 8 -> 8 groups of n=512 tokens) -> out proj.

Sharding: by (batch, residue-group).  B*stride = 16 group-instances; each of
the 8 cores owns 2 (batch,group) pairs = 1024 tokens and computes their FULL
output rows (it holds all 16 heads for its tokens).  The residue groups are
independent, so there are no cross-device collectives at all; the host
permutes tokens into group-major order on the way in and inverts on the way
out.

Device kernel design (per core), v2:
  - All inputs host-pre-laid-out partition-contiguous; ONE large DMA per
    tensor, issued from five different engine queues in parallel so the
    sequencer issue cost (~1us per DMA) never serializes.  Weights are
    SBUF-resident and loaded exactly once (v1 loaded wq/wk/wv per group).
  - QKV: qT,kT produced feature-on-partition ([f,tok]); v produced
    token-on-partition ([tok,f]).  fp16 matmuls.
  - scoresT[k,q] = kT.T-chunks @ qT per head; head pairs are row-packed on
    the PE array (K=64 each at array rows 0-63 / 64-127).
  - exp on ScalarE without max-subtraction (scores are O(+-8), exp is safe).
    The EXP act table is loaded once and never evicted (softmax recip no
    longer uses ScalarE -- v1's ln/exp recip thrashed the table).
  - PV: lhsT = [v | ones] (even heads) or [ones | v] (odd heads) so one
    accumulation chain yields both o-rows and 64 replicated softmax
    denominator rows.  The ones pattern comes from a whole-tile memset
    (v halves are overwritten by the v projection).
  - softmax normalize: DVE reciprocal on the denominator row, a stride-0
    partition-broadcast DMA to mirror it across the partition halves, DVE
    multiply.  Emitted per head-pair, pipelined inside the attention loops
    so the out projection of a group can start the moment its last pair
    finishes.
  - out proj: lhsT = oT chunks, rhs = Wout rows -> natural [tok, E] output.
"""

import os

import numpy as np

B, S, E = 2, 4096, 1024
H, ST = 16, 8
DH = E // H  # 64
N = S // ST  # 512 tokens per residue group
NCORES = 8
GPC = (B * ST) // NCORES  # 2 (batch,group) pairs per core
TOK = GPC * N  # 1024 tokens per core
P = 128
EC = E // P  # 8 contraction chunks of 128
NB = N // P  # 4 token chunks of 128 per group
FB = 2  # feature blocks of 512 in E
SCALE = 1.0 / float(np.sqrt(DH))

_CACHE: dict = {}


def _build_nc():
    import concourse.bass as bass
    import concourse.bacc as bacc
    import concourse.tile as tile
    from concourse import mybir

    F32 = mybir.dt.float32
    FP16 = mybir.dt.float16
    ADD = mybir.AluOpType.add
    EXP = mybir.ActivationFunctionType.Exp

    nc = bacc.Bacc()
    xt = nc.declare_dram_parameter("xt", [P, EC, TOK], FP16, isOutput=False)
    wq = nc.declare_dram_parameter("wq", [P, EC, EC, P], FP16, isOutput=False)
    wk = nc.declare_dram_parameter("wk", [P, EC, EC, P], FP16, isOutput=False)
    wv = nc.declare_dram_parameter("wv", [P, EC, E], FP16, isOutput=False)
    wo = nc.declare_dram_parameter("wo", [P, FB, EC, 512], FP16, isOutput=False)
    bq = nc.declare_dram_parameter("bq", [E], F32, isOutput=False)
    bk = nc.declare_dram_parameter("bk", [E], F32, isOutput=False)
    bv = nc.declare_dram_parameter("bv", [P, E], F32, isOutput=False)
    bo = nc.declare_dram_parameter("bo", [P, E], F32, isOutput=False)
    out = nc.declare_dram_parameter("out", [TOK, E], F32, isOutput=True)

    with tile.TileContext(nc) as tc, (
        tc.tile_pool(name="const", bufs=1)
    ) as const, tc.tile_pool(name="qtp", bufs=9) as qtp, tc.tile_pool(
        name="ktp", bufs=9
    ) as ktp, tc.tile_pool(name="vpp", bufs=5) as vpp, tc.tile_pool(
        name="expp", bufs=3
    ) as expp, tc.tile_pool(name="otp", bufs=17) as otp, tc.tile_pool(
        name="recp", bufs=4
    ) as recp, tc.tile_pool(name="outp", bufs=3) as outp, tc.tile_pool(
        name="osbp", bufs=8
    ) as osbp, tc.tile_pool(name="psmm", bufs=2, space="PSUM") as psmm, tc.tile_pool(
        name="pssc", bufs=2, space="PSUM"
    ) as pssc, tc.tile_pool(name="pso", bufs=2, space="PSUM") as psop:
        # ---- input loads: one big DMA per tensor, five issue queues ----
        wq_sb = const.tile([P, EC, EC, P], FP16)
        nc.sync.dma_start(out=wq_sb, in_=wq[:])
        xt_sb = const.tile([P, EC, TOK], FP16)
        nc.scalar.dma_start(out=xt_sb, in_=xt[:])
        wk_sb = const.tile([P, EC, EC, P], FP16)
        nc.gpsimd.dma_start(out=wk_sb, in_=wk[:])
        wv_sb = const.tile([P, EC, E], FP16)
        nc.sync.dma_start(out=wv_sb, in_=wv[:])
        wo_sb = const.tile([P, FB, EC, 512], FP16)
        nc.scalar.dma_start(out=wo_sb, in_=wo[:])
        bq_sb = const.tile([P, EC], F32)
        nc.sync.dma_start(out=bq_sb, in_=bq[:].rearrange("(c p) -> p c", p=P))
        bk_sb = const.tile([P, EC], F32)
        nc.sync.dma_start(out=bk_sb, in_=bk[:].rearrange("(c p) -> p c", p=P))
        bv_sb = const.tile([P, E], F32)
        nc.gpsimd.dma_start(out=bv_sb, in_=bv[:])
        bo_sb = const.tile([P, E], F32)
        nc.gpsimd.dma_start(out=bo_sb, in_=bo[:])

        osbs = {0: {}, 1: {}}
        qts = {0: [], 1: []}
        kts = {0: [], 1: []}
        vts = {0: [], 1: []}
        ots = {0: [], 1: []}

        def emit_qk_ftile(g, which, ft):
            w_sb, bias_sb, pool, lst, tag = (
                (wq_sb, bq_sb, qtp, qts[g], "qt")
                if which == "q"
                else (wk_sb, bk_sb, ktp, kts[g], "kt")
            )
            ps = psmm.tile([P, N], F32, tag="mm")
            for c in range(EC):
                nc.tensor.matmul(
                    ps,
                    lhsT=w_sb[:, ft, c, :],
                    rhs=xt_sb[:, c, g * N : (g + 1) * N],
                    start=(c == 0),
                    stop=(c == EC - 1),
                )
            t = pool.tile([P, N], FP16, tag=tag)
            nc.vector.tensor_scalar(
                out=t, in0=ps, scalar1=bias_sb[:, ft : ft + 1], scalar2=None, op0=ADD
            )
            lst.append(t)

        def emit_v_fb(g, fb):
            if fb == 0:
                for tt in range(NB):
                    vt = vpp.tile([P, H, P], FP16, tag="vp")
                    # whole-tile ones; the v projection overwrites the v
                    # halves, leaving the denominator ones pattern
                    nc.vector.memset(vt[:], 1.0)
                    vts[g].append(vt)
            for tt in range(NB):
                ps = psmm.tile([P, 512], F32, tag="mm")
                for c in range(EC):
                    nc.tensor.matmul(
                        ps,
                        lhsT=xt_sb[:, c, g * N + tt * P : g * N + (tt + 1) * P],
                        rhs=wv_sb[:, c, fb * 512 : (fb + 1) * 512],
                        start=(c == 0),
                        stop=(c == EC - 1),
                    )
                for hl in range(8):
                    h = fb * 8 + hl
                    off = 0 if (h % 2 == 0) else DH
                    nc.vector.tensor_add(
                        out=vts[g][tt][:, h, off : off + DH],
                        in0=ps[:, hl * DH : (hl + 1) * DH],
                        in1=bv_sb[:, fb * 512 + hl * DH : fb * 512 + (hl + 1) * DH],
                    )

        def emit_attn_pair(g, pr):
            # scores for both heads of the pair, row-packed on the PE array
            # (K=64 each at array rows 0-63 / 64-127, separate PSUM banks)
            ex_AB = {}
            for h in (2 * pr, 2 * pr + 1):
                ex_AB[h] = expp.tile([P, NB, N], FP16, tag="exp", name=f"ex{h}")
            for half in range(2):
                scs = {}
                for h in (2 * pr, 2 * pr + 1):
                    lo, hi = (0, DH) if h % 2 == 0 else (DH, P)
                    sc = pssc.tile([P, 2, N], F32, tag="sc")
                    for cc in range(2):
                        c = 2 * half + cc
                        nc.tensor.matmul(
                            sc[:, cc],
                            lhsT=kts[g][pr][lo:hi, c * P : (c + 1) * P],
                            rhs=qts[g][pr][lo:hi, :],
                            start=True,
                            stop=True,
                        )
                    scs[h] = sc
                for h in (2 * pr, 2 * pr + 1):
                    nc.scalar.activation(
                        out=ex_AB[h][:, 2 * half : 2 * half + 2],
                        in_=scs[h],
                        func=EXP,
                    )
            for h in (2 * pr, 2 * pr + 1):
                ex = ex_AB[h]
                po = psop.tile([P, N], F32, tag="po")
                for c in range(NB):
                    nc.tensor.matmul(
                        po,
                        lhsT=vts[g][c][:, h, :],
                        rhs=ex[:, c, :],
                        start=(c == 0),
                        stop=(c == NB - 1),
                    )
                # evacuate PSUM (unnormalized o + replicated denominator rows)
                osb = osbp.tile([P, N], FP16, tag="osb")
                nc.vector.tensor_copy(out=osb, in_=po)
                osbs[g][h] = osb

        def emit_recip_pair(g, pr):
            # softmax normalize, all on DVE + one small broadcast DMA per
            # head; no ScalarE involvement so the EXP act table stays loaded
            ot = otp.tile([P, N], FP16, tag="ot")
            for h in (2 * pr, 2 * pr + 1):
                osb = osbs[g][h]
                rec = recp.tile([P, N], F32, tag="rec")
                rec2 = recp.tile([P, N], F32, tag="rec2")
                if h % 2 == 0:
                    # o rows 0-63; replicated denominator rows 64-127.  A
                    # stride-0 partition-broadcast DMA mirrors the recip row
                    # across the halves (engines cannot cross partitions).
                    nc.vector.reciprocal(
                        out=rec[DH : DH + 1, :], in_=osb[DH : DH + 1, :]
                    )
                    s = rec[DH : DH + 1, :]
                    nc.scalar.dma_start(
                        out=rec2[0:DH, :],
                        in_=bass.AP(
                            tensor=s.tensor,
                            offset=s.offset,
                            ap=[list(s.ap[0]), [0, DH], list(s.ap[1])],
                        ),
                    )
                    nc.vector.tensor_mul(
                        out=ot[0:DH, :], in0=osb[0:DH, :], in1=rec2[0:DH, :]
                    )
                else:
                    # denominator rows 0-63, o rows 64-127
                    nc.vector.reciprocal(out=rec[0:1, :], in_=osb[0:1, :])
                    s = rec[0:1, :]
                    nc.gpsimd.dma_start(
                        out=rec2[DH:P, :],
                        in_=bass.AP(
                            tensor=s.tensor,
                            offset=s.offset,
                            ap=[list(s.ap[0]), [0, DH], list(s.ap[1])],
                        ),
                    )
                    nc.vector.tensor_mul(
                        out=ot[DH:P, :], in0=osb[DH:P, :], in1=rec2[DH:P, :]
                    )
            ots[g].append(ot)

        def emit_outproj_unit(g, fb, tt):
            ps = psmm.tile([P, 512], F32, tag="mm")
            for dc in range(EC):
                nc.tensor.matmul(
                    ps,
                    lhsT=ots[g][dc][:, tt * P : (tt + 1) * P],
                    rhs=wo_sb[:, fb, dc, :],
                    start=(dc == 0),
                    stop=(dc == EC - 1),
                )
            ob = outp.tile([P, 512], F32, tag="ob")
            nc.vector.tensor_add(
                out=ob, in0=ps, in1=bo_sb[:, fb * 512 : (fb + 1) * 512]
            )
            nc.sync.dma_start(
                out=out[
                    g * N + tt * P : g * N + (tt + 1) * P,
                    fb * 512 : (fb + 1) * 512,
                ],
                in_=ob,
            )

        # ---- software-pipelined program order --------------------------
        for ft in range(EC):
            emit_qk_ftile(0, "q", ft)
        for ft in range(EC):
            emit_qk_ftile(0, "k", ft)
        for fb in range(FB):
            emit_v_fb(0, fb)
        # group-0 attention interleaved with group-1 q/k proj; each pair's
        # softmax normalize rides one step behind its attention
        for pr in range(EC):
            emit_attn_pair(0, pr)
            if pr > 0:
                emit_recip_pair(0, pr - 1)
            emit_qk_ftile(1, "q", pr)
            emit_qk_ftile(1, "k", pr)
        emit_recip_pair(0, EC - 1)
        for fb in range(FB):
            emit_v_fb(1, fb)
        # group-1 attention interleaved with group-0 out proj
        for pr in range(EC):
            emit_attn_pair(1, pr)
            if pr > 0:
                emit_recip_pair(1, pr - 1)
            fb, tt = pr // 4, pr % 4
            emit_outproj_unit(0, fb, tt)
        emit_recip_pair(1, EC - 1)
        # group-1 out proj: all inputs ready, dense PE phase
        for fb in range(FB):
            for tt in range(NB):
                emit_outproj_unit(1, fb, tt)
    nc.finalize()
    return nc


def _get_nc():
    if "nc" not in _CACHE:
        _CACHE["nc"] = _build_nc()
    return _CACHE["nc"]


def _make_in_maps(x, Wqkv, bqkv, Wout, bout):
    """Host-side sharding: permute tokens to group-major, pre-transpose x,
    and lay every weight out partition-contiguous for single-DMA loads."""
    x = np.asarray(x, dtype=np.float32)
    Wqkv = np.asarray(Wqkv, dtype=np.float32)
    bqkv = np.asarray(bqkv, dtype=np.float32)
    Wout = np.asarray(Wout, dtype=np.float32)
    bout = np.asarray(bout, dtype=np.float32)

    # group-major token order: x_perm[b, g*N + i] = x[b, i*ST + g]
    x_perm = x.reshape(B, N, ST, E).transpose(0, 2, 1, 3)  # [B, ST, N, E]

    # [E_in, E_out] -> [p, ft, c, m] with w_l[p, ft, c, m] = W[c*P+p, ft*P+m]
    def layout_qk(w):
        return np.ascontiguousarray(
            w.reshape(EC, P, EC, P).transpose(1, 2, 0, 3).astype(np.float16)
        )

    wq = layout_qk(Wqkv[:, 0:E] * SCALE)
    wk = layout_qk(Wqkv[:, E : 2 * E])
    # [E_in, E_out] -> [p, c, n] with wv_l[p, c, n] = Wv[c*P+p, n]
    wv = np.ascontiguousarray(
        Wqkv[:, 2 * E : 3 * E].reshape(EC, P, E).transpose(1, 0, 2).astype(np.float16)
    )
    # [E_in, E_out] -> [p, fb, dc, n] with wo_l[p, fb, dc, n] = Wo[dc*P+p, fb*512+n]
    wo = np.ascontiguousarray(
        Wout.reshape(EC, P, FB, 512).transpose(1, 2, 0, 3).astype(np.float16)
    )
    bq = np.ascontiguousarray(bqkv[0:E] * SCALE)
    bk = np.ascontiguousarray(bqkv[E : 2 * E])
    bv = np.ascontiguousarray(np.broadcast_to(bqkv[2 * E : 3 * E], (P, E)))
    bo = np.ascontiguousarray(np.broadcast_to(bout, (P, E)))

    in_maps = []
    for c in range(NCORES):
        b = c // (NCORES // B)
        g0 = GPC * (c % (NCORES // B))
        xc = x_perm[b, g0 : g0 + GPC].reshape(TOK, E)  # [1024, E]
        xct = xc.T.astype(np.float16)  # [E, 1024]
        xtl = np.ascontiguousarray(
            xct.reshape(EC, P, TOK).transpose(1, 0, 2)
        )  # [p, c, t]
        in_maps.append(
            {
                "xt": xtl,
                "wq": wq,
                "wk": wk,
                "wv": wv,
                "wo": wo,
                "bq": bq,
                "bk": bk,
                "bv": bv,
                "bo": bo,
            }
        )
    return in_maps


def kernel(x, Wqkv, bqkv, Wout, bout):
    from concourse.bass_utils import run_bass_kernel_spmd

    nc = _get_nc()
    in_maps = _make_in_maps(x, Wqkv, bqkv, Wout, bout)
    trace = bool(int(os.environ.get("KERNEL_TRACE", "0")))
    res = run_bass_kernel_spmd(
        nc, in_maps, core_ids=list(range(NCORES)), trace=trace
    )
    _CACHE["last_result"] = res

    # reassemble: core outputs are [1024 tok, E] in group-major token order
    out = np.empty((B, S, E), dtype=np.float32)
    for b in range(B):
        per_b = [res.results[b * (NCORES // B) + j]["out"] for j in range(NCORES // B)]
        perm = np.concatenate(per_b, axis=0)  # [ST*N, E] group-major
        out[b] = perm.reshape(ST, N, E).transpose(1, 0, 2).reshape(S, E)
    return out
